# revision 14
# baseline (speedup 1.0000x reference)
"""Policy-masked sparse attention on 8 trn2 NeuronCores.

Strategy (data-parallel over B: one batch element per core):
  The reference softmax-with-policy (eps=1e-6) reduces, for this input
  regime, to:
    - dropped queries (policy=0): out row = v_row exactly (rel err ~1e-5)
    - kept queries: out row = (E @ V') / (E @ pol), E = exp(S), over kept
      keys only (diagonal is included since a kept query is a kept key)
  Scores are small (|S| < ~3) so exp needs no row-max subtraction
  (shift-invariance holds once eps is negligible).

  Host side: compact kept/dropped token indices per batch (counts ~700/
  ~320), pad to multiples of 128, pre-transpose x, pre-scale Wq by
  1/sqrt(hd), fold Wproj@Wv for the dropped path, cast operands fp16.

  Device side per core (all matmuls fp16 operands, fp32 PSUM):
    S^T = K^T.T @ Q^T in [key, query] layout -> exp on ScalarE -> E^T ->
    T^T = [V|pol].T @ E^T in [head_dim+1, query] PSUM (V stays the
    stationary operand so E^T streams at full rate) -> row 64 is the
    softmax denominator: reciprocal_approx_fast + gpsimd
    partition_broadcast -> one tensor_tensor multiply normalizes and
    writes the feature-major attention output -> proj matmul; dropped
    tokens get x_d @ (Wproj@Wv)^T directly. Host scatters rows back.
"""

import math
import numpy as np

import concourse.bass as bass
import concourse.bacc as bacc
import concourse.mybir as mybir
from concourse import tile
from concourse.bass_utils import run_bass_kernel_spmd

C = 768
H = 12
HD = 64
CB = C // 128          # feature blocks of 128
F16 = mybir.dt.float16
F32 = mybir.dt.float32

_cache = {}


def _groups(n, limit=512):
    out = []
    off = 0
    while off < n:
        g = min(limit, n - off)
        out.append((off, g))
        off += g
    return out


def _build(NK, ND, NKM):
    """Build + bacc-compile the 8-core SPMD program for padded sizes."""
    KB = NK // 128
    DB = ND // 128
    VW = 66                      # per-head stride in V_aug: 64 v + 1 pol + 1 pad
    nc = bacc.Bacc("TRN2", target_bir_lowering=False, debug=False,
                   num_devices=8)

    xcT = nc.dram_tensor("xcT", [C, NK], F16, kind="ExternalInput").ap()
    xdT = nc.dram_tensor("xdT", [C, ND], F16, kind="ExternalInput").ap()
    wqkvT = nc.dram_tensor("wqkvT", [C, 3 * C], F16, kind="ExternalInput").ap()
    wprojT = nc.dram_tensor("wprojT", [C, C], F16, kind="ExternalInput").ap()
    w2T = nc.dram_tensor("w2T", [C, C], F16, kind="ExternalInput").ap()
    polb = nc.dram_tensor("polb", [NK, H], F16, kind="ExternalInput").ap()
    biasb = nc.dram_tensor("biasb", [128, C], F32, kind="ExternalInput").ap()
    outk = nc.dram_tensor("outk", [NK, C], F16, kind="ExternalOutput").ap()
    outd = nc.dram_tensor("outd", [ND, C], F16, kind="ExternalOutput").ap()

    GK = _groups(NK)             # moving-dim groups over kept tokens
    GKM = _groups(NKM)           # moving-dim groups over real kept queries
    GC = _groups(C)              # moving-dim groups over features

    with tile.TileContext(nc) as tc:
        with (
            tc.tile_pool(name="const", bufs=1) as cpool,
            tc.tile_pool(name="ins", bufs=1) as ipool,
            tc.tile_pool(name="acts", bufs=1) as apool,
            tc.tile_pool(name="work", bufs=4) as wpool,
            tc.tile_pool(name="outs", bufs=3) as opool,
            tc.tile_pool(name="ps", bufs=2, space="PSUM") as pspool,
            tc.tile_pool(name="pt", bufs=2, space="PSUM") as ptpool,
        ):
            # ---- inputs (DMA emission order = priority order) ----
            wq_t = [ipool.tile([128, 3 * C], F16, name=f"wq{cb}",
                               tag=f"wq{cb}") for cb in range(CB)]
            xc_t = [ipool.tile([128, NK], F16, name=f"xc{cb}",
                               tag=f"xc{cb}") for cb in range(CB)]
            # DMA priority: K weights + x first (unblocks qkv -> S),
            # then Q, V weights, then the dropped-path and proj weights.
            for cb in range(CB):
                nc.sync.dma_start(wq_t[cb][:, C:2 * C],
                                  wqkvT[cb * 128:(cb + 1) * 128, C:2 * C])
            for cb in range(CB):
                nc.sync.dma_start(xc_t[cb][:], xcT[cb * 128:(cb + 1) * 128, :])
            for cb in range(CB):
                nc.sync.dma_start(wq_t[cb][:, 0:C],
                                  wqkvT[cb * 128:(cb + 1) * 128, 0:C])
            for cb in range(CB):
                nc.sync.dma_start(wq_t[cb][:, 2 * C:3 * C],
                                  wqkvT[cb * 128:(cb + 1) * 128, 2 * C:3 * C])
            pol_t = []
            for tb in range(KB):
                t = ipool.tile([128, H], F16, name=f"pol{tb}", tag=f"pol{tb}")
                nc.sync.dma_start(t[:], polb[tb * 128:(tb + 1) * 128, :])
                pol_t.append(t)
            xd_t = []
            w2_t = []
            for cb in range(CB):
                t = ipool.tile([128, ND], F16, name=f"xd{cb}", tag=f"xd{cb}")
                nc.sync.dma_start(t[:], xdT[cb * 128:(cb + 1) * 128, :])
                xd_t.append(t)
            for cb in range(CB):
                t2 = ipool.tile([128, C], F16, name=f"w2{cb}", tag=f"w2{cb}")
                nc.sync.dma_start(t2[:], w2T[cb * 128:(cb + 1) * 128, :])
                w2_t.append(t2)
            bias_t = cpool.tile([128, C], F32, name="bias", tag="bias")
            nc.sync.dma_start(bias_t[:], biasb[:])
            wp_t = []
            for cb in range(CB):
                t = ipool.tile([128, C], F16, name=f"wp{cb}", tag=f"wp{cb}")
                nc.sync.dma_start(t[:], wprojT[cb * 128:(cb + 1) * 128, :])
                wp_t.append(t)

            # ---- persistent intermediates ----
            QcT = [apool.tile([128, NK], F16, name=f"q{j}", tag=f"q{j}")
                   for j in range(CB)]
            KcT = [apool.tile([128, NK], F16, name=f"k{j}", tag=f"k{j}")
                   for j in range(CB)]
            Vag = [apool.tile([128, H * VW], F16, name=f"va{tb}",
                              tag=f"va{tb}") for tb in range(KB)]
            OAT = [apool.tile([128, NK], F16, name=f"oat{j}", tag=f"oat{j}")
                   for j in range(CB)]
            for j in range(CB):
                nc.gpsimd.memset(OAT[j][:], 0.0)
            ET = {}
            for hm in range(4):
                for kb in range(KB):
                    ET[(hm, kb)] = apool.tile(
                        [128, NK], F16, name=f"et{hm}_{kb}",
                        tag=f"et{hm}_{kb}")

            def qkv_chunk(j):
                """f-major chunk j of Wqkv (j in 0..11 -> Q/K).

                Queries beyond the real kept count are never read, so Q
                chunks only compute NKM columns; K chunks need all NK
                (keys are contraction inputs and must be zero-padded).
                """
                grps = GKM if j < CB else GK
                w = NKM if j < CB else NK
                ps = pspool.tile([128, NK], F32, name="qps", tag="s")
                for cb in range(CB):
                    for (o, n) in grps:
                        nc.tensor.matmul(
                            ps[:, o:o + n],
                            lhsT=wq_t[cb][:, j * 128:(j + 1) * 128],
                            rhs=xc_t[cb][:, o:o + n],
                            start=(cb == 0), stop=(cb == CB - 1))
                dest = QcT[j] if j < CB else KcT[j - CB]
                nc.vector.tensor_copy(dest[:, 0:w], ps[:, 0:w])

            def v_chunk(tb):
                """token-major V chunk for kept token block tb."""
                ps = pspool.tile([128, C], F32, name="vps", tag="s")
                for cb in range(CB):
                    for (o, n) in GC:
                        nc.tensor.matmul(
                            ps[:, o:o + n],
                            lhsT=xc_t[cb][:, tb * 128:(tb + 1) * 128],
                            rhs=wq_t[cb][:, 2 * C + o:2 * C + o + n],
                            start=(cb == 0), stop=(cb == CB - 1))
                va = Vag[tb]
                va3 = va[:].rearrange("p (h s) -> p h s", s=VW)
                ps3 = ps[:].rearrange("p (h s) -> p h s", s=HD)
                nc.vector.tensor_copy(va3[:, :, 0:HD], ps3)
                pol3 = pol_t[tb][:].rearrange("p (h o) -> p h o", o=1)
                nc.vector.tensor_copy(va3[:, :, HD:HD + 1], pol3)

            def s_exp_kb(p, kb):
                """S^T then exp for both heads of pair p at key block kb."""
                fc = p
                for hh in range(2):
                    h = 2 * p + hh
                    rows = slice(hh * 64, hh * 64 + 64)
                    et = ET[(h % 4, kb)]
                    ps = pspool.tile([128, NKM], F32, name="sps", tag="s")
                    for (o, n) in GKM:
                        nc.tensor.matmul(
                            ps[:, o:o + n],
                            lhsT=KcT[fc][rows, kb * 128:(kb + 1) * 128],
                            rhs=QcT[fc][rows, o:o + n],
                            start=True, stop=True)
                    nc.scalar.activation(
                        et[:, 0:NKM], ps[:],
                        mybir.ActivationFunctionType.Exp)

            def tt_kb(p, kb, ptTs):
                """Accumulate T^T += V_aug.T @ E^T for both heads at kb."""
                for hh in range(2):
                    h = 2 * p + hh
                    et = ET[(h % 4, kb)]
                    for (o, n) in GKM:
                        nc.tensor.matmul(
                            ptTs[hh][:, o:o + n],
                            lhsT=Vag[kb][:, h * VW:h * VW + 65],
                            rhs=et[:, o:o + n],
                            start=(kb == 0), stop=(kb == KB - 1))

            def t_finalize(p, ptTs, split_norm=False):
                """Normalize T^T rows by row 64 and write OAT (fp16)."""
                for hh in range(2):
                    h = 2 * p + hh
                    cf = h // 2
                    orow = (h % 2) * 64
                    ptT = ptTs[hh]
                    s_sb = wpool.tile([1, NKM], F32, name="srow", tag="srow")
                    nc.scalar.copy(s_sb[:], ptT[64:65, :])
                    r_sb = wpool.tile([1, NKM], F32, name="rrow", tag="rrow")
                    nc.vector.reciprocal_approx_fast(r_sb[:], s_sb[:])
                    rb = wpool.tile([64, NKM], F32, name="rb", tag="rb")
                    nc.gpsimd.partition_broadcast(rb[:], r_sb[:], channels=64)
                    if split_norm:
                        # finer writes let the tail proj start per t-chunk
                        for tb in range((NKM + 127) // 128):
                            cols = slice(tb * 128, min((tb + 1) * 128, NKM))
                            nc.vector.tensor_tensor(
                                OAT[cf][orow:orow + 64, cols],
                                ptT[0:64, cols], rb[:, cols],
                                op=mybir.AluOpType.mult)
                    else:
                        nc.vector.tensor_tensor(
                            OAT[cf][orow:orow + 64, 0:NKM], ptT[0:64, :],
                            rb[:], op=mybir.AluOpType.mult)

            def proj_kept(tb):
                ps = pspool.tile([128, C], F32, name="pps", tag="s")
                for fb in range(CB):
                    for (o, n) in GC:
                        nc.tensor.matmul(
                            ps[:, o:o + n],
                            lhsT=OAT[fb][:, tb * 128:(tb + 1) * 128],
                            rhs=wp_t[fb][:, o:o + n],
                            start=(fb == 0), stop=(fb == CB - 1))
                ok = opool.tile([128, C], F16, name="ok", tag="ok")
                nc.vector.tensor_add(ok[:], ps[:], bias_t[:])
                nc.sync.dma_start(outk[tb * 128:(tb + 1) * 128, :], ok[:])

            def proj_drop(td):
                ps = pspool.tile([128, C], F32, name="dps", tag="s")
                for cb in range(CB):
                    for (o, n) in GC:
                        nc.tensor.matmul(
                            ps[:, o:o + n],
                            lhsT=xd_t[cb][:, td * 128:(td + 1) * 128],
                            rhs=w2_t[cb][:, o:o + n],
                            start=(cb == 0), stop=(cb == CB - 1))
                ok = opool.tile([128, C], F16, name="ok", tag="ok")
                nc.vector.tensor_add(ok[:], ps[:], bias_t[:])
                nc.sync.dma_start(outd[td * 128:(td + 1) * 128, :], ok[:])

            # ---- schedule ----
            qkv_chunk(CB + 0)      # K pair 0
            qkv_chunk(0)           # Q pair 0
            NP = H // 2
            for p in range(NP):
                ptTs = [ptpool.tile([65, NKM], F32, name=f"ptT{hh}",
                                    tag="t2") for hh in range(2)]
                for kb in range(KB):
                    if p == 0:
                        v_chunk(kb)
                    s_exp_kb(p, kb)
                    tt_kb(p, kb, ptTs)
                if p + 1 < NP:
                    qkv_chunk(CB + p + 1)
                    qkv_chunk(p + 1)
                t_finalize(p, ptTs, split_norm=(p == NP - 1))
            for td in range(DB):
                proj_drop(td)      # independent: fills the tail's idle
            for tb in range(KB):
                proj_kept(tb)

    nc.compile()
    return nc


def kernel(x, policy, Wqkv, Wproj, bproj, _trace=False, _tmpdir=None):
    x = np.asarray(x)
    policy = np.asarray(policy)
    Wqkv = np.asarray(Wqkv, dtype=np.float32)
    Wproj = np.asarray(Wproj, dtype=np.float32)
    bproj = np.asarray(bproj, dtype=np.float32)
    B, N, _ = x.shape
    assert B == 8 and x.shape[2] == C

    pol = policy[:, :, 0] > 0.5
    kept = [np.nonzero(pol[b])[0] for b in range(B)]
    drop = [np.nonzero(~pol[b])[0] for b in range(B)]
    nk = [len(i) for i in kept]
    nd = [len(i) for i in drop]
    NK = max(128, int(math.ceil(max(nk) / 128.0)) * 128)
    ND = max(128, int(math.ceil(max(nd) / 128.0)) * 128)
    NKM = min(NK, max(128, int(math.ceil(max(nk) / 32.0)) * 32))

    key = (NK, ND, NKM)
    if key not in _cache:
        _cache[key] = _build(NK, ND, NKM)
    nc = _cache[key]

    # shared weight prep
    wqkv_s = Wqkv.copy()
    wqkv_s[:C] *= HD ** -0.5                 # fold attention scale into Wq
    wqkvT = np.ascontiguousarray(wqkv_s.T).astype(np.float16)
    wprojT = np.ascontiguousarray(Wproj.T).astype(np.float16)
    W2 = Wproj @ Wqkv[2 * C:3 * C]
    w2T = np.ascontiguousarray(W2.T).astype(np.float16)
    biasb = np.ascontiguousarray(
        np.broadcast_to(bproj[None, :], (128, C))).astype(np.float32)

    in_maps = []
    for b in range(B):
        xcTa = np.zeros((C, NK), np.float16)
        xcTa[:, :nk[b]] = x[b][kept[b]].T
        xdTa = np.zeros((C, ND), np.float16)
        xdTa[:, :nd[b]] = x[b][drop[b]].T
        polba = np.zeros((NK, H), np.float16)
        polba[:nk[b], :] = 1.0
        in_maps.append({
            "xcT": xcTa, "xdT": xdTa, "wqkvT": wqkvT, "wprojT": wprojT,
            "w2T": w2T, "polb": polba, "biasb": biasb,
        })

    res = run_bass_kernel_spmd(nc, in_maps, core_ids=list(range(B)),
                               trace=_trace, tmpdir=_tmpdir)

    out = np.empty((B, N, C), np.float32)
    for b in range(B):
        out[b, kept[b]] = res.results[b]["outk"][:nk[b]].astype(np.float32)
        out[b, drop[b]] = res.results[b]["outd"][:nd[b]].astype(np.float32)
    if _trace:
        kernel._last = res
    return out
